# revision 1
# baseline (speedup 1.0000x reference)
"""Trainium2 Bass kernel for nn_Cst2Val_Layer (GNN message passing).

Strategy (8 NeuronCores):
  - Shard constraints (cst) and values (val) row-wise: core c owns cst rows
    [c*NCs, (c+1)*NCs) and val rows [c*NVs, (c+1)*NVs).
  - Edges are sharded by SOURCE cst, so each core computes its cst_send MLP
    shard m = LN(ReLU(r_cst@W1+b1)@W2) locally (stored to HBM as [4*NCs, H]
    rows), gathers its edges' message rows with dma_gather, and scatter-adds
    them into a full-size partial r_val accumulator with dma_scatter_add
    (SDMA CCE add).  Scatter calls are built as conflict-free rounds (each
    destination row at most once per call) to avoid CCE read-modify-write
    races; Tile's WAW tracking serializes rounds of the same chunk.
  - The partial r_val is reduce-scattered across the 8 cores in J chunks
    (laid out so each RS chunk hands every core a contiguous slice of its
    own val shard), overlapping the collective with the remaining edge work.
  - val_rec MLP + LayerNorm + residual runs on the local val shard.
"""

import math
from dataclasses import dataclass

import numpy as np

H = 128
EPS = 1e-5


@dataclass(frozen=True)
class Cfg:
    cores: int = 8
    nc_tot: int = 100000
    nv_tot: int = 100000
    J: int = 4          # number of reduce-scatter chunks
    sub: int = 1024     # max slots per gather/scatter call (HW limit: >=1280 wedges)

    @property
    def NCs(self):
        return self.nc_tot // self.cores

    @property
    def T1(self):
        t = -(-self.NCs // 128)
        assert t % 2 == 0, "T1 must be even for the 2-bank m split"
        return t

    @property
    def NCp(self):
        return self.T1 * 128

    @property
    def BANK_TILES(self):
        return self.T1 // 2

    @property
    def BANK_ROWS(self):
        return self.BANK_TILES * 512

    @property
    def BANK_ALLOC(self):
        return self.BANK_ROWS + 128  # trailing zero rows (gather pad target)

    @property
    def NVs(self):
        return self.nv_tot // self.cores

    @property
    def S(self):
        assert self.NVs % self.J == 0
        return self.NVs // self.J

    @property
    def CH_ROWS(self):
        return self.cores * self.S

    @property
    def CH_ALLOC(self):
        # +dummy rows (scatter pad target), round to 128
        return -(-(self.CH_ROWS + 1) // 128) * 128

    @property
    def T2(self):
        return -(-self.NVs // 128)

    @property
    def NVp(self):
        return self.T2 * 128


def _round_up(x, m):
    return -(-x // m) * m


def _occurrence_rank(keys):
    """For a SORTED int array, return the occurrence index of each element
    within its run of equal values."""
    n = keys.shape[0]
    if n == 0:
        return np.zeros(0, np.int64)
    new_run = np.empty(n, bool)
    new_run[0] = True
    np.not_equal(keys[1:], keys[:-1], out=new_run[1:])
    run_id = np.cumsum(new_run) - 1
    run_start = np.flatnonzero(new_run)
    return np.arange(n, dtype=np.int64) - run_start[run_id]


def _build_plan(cfg, src, dst, slot):
    """Host-side edge preprocessing.

    Returns (calls, tot_slots, gstream, sstream) where
      calls: per chunk j, list of scatter-call windows
             (slot0, length, [(gslot0, glen, bank), ...])
      gstream/sstream: [cores, tot_slots] int16 gather/scatter index streams.
    """
    C, J, S = cfg.cores, cfg.J, cfg.S
    E = src.shape[0]

    core = src // cfg.NCs
    m_row = 4 * (src % cfg.NCs) + slot          # [0, 4*NCs)
    bank = m_row // cfg.BANK_ROWS               # 0/1
    row_in_bank = m_row % cfg.BANK_ROWS

    dr = dst // cfg.NVs
    off = dst % cfg.NVs
    j = off // S
    k = off % S
    scat_local = dr * S + k                     # [0, CH_ROWS)

    # conflict-free round number: occurrence rank of (core, j, scat_local)
    comb = (core * J + j) * cfg.CH_ROWS + scat_local
    order = np.argsort(comb, kind="stable")
    rank_sorted = _occurrence_rank(comb[order])
    rank = np.empty(E, np.int64)
    rank[order] = rank_sorted

    # segment = (j, r, s); per-core counts -> capped segment sizes
    n_r = int(rank.max()) + 1 if E else 1
    seg_of_edge = (j * n_r + rank) * 2 + bank
    n_seg = J * n_r * 2
    counts = np.zeros((C, n_seg), np.int64)
    np.add.at(counts, (core, seg_of_edge), 1)
    cap = counts.max(axis=0)                    # [n_seg]
    cap = np.where(cap > 0, _round_up(np.maximum(cap, 1), 128), 0)

    # lay out segments: per chunk j, rounds in order, banks in order
    seg_base = np.zeros(n_seg, np.int64)
    calls = [[] for _ in range(J)]
    pos = 0
    for jj in range(J):
        for r in range(n_r):
            s_ids = [(jj * n_r + r) * 2 + 0, (jj * n_r + r) * 2 + 1]
            lens = [int(cap[s_ids[0]]), int(cap[s_ids[1]])]
            if lens[0] + lens[1] == 0:
                continue
            round_base = pos
            seg_base[s_ids[0]] = pos
            seg_base[s_ids[1]] = pos + lens[0]
            L = lens[0] + lens[1]
            pos += L
            # windows of <= sub slots; all boundaries are multiples of 128
            w0 = 0
            while w0 < L:
                wl = min(cfg.sub, L - w0)
                gathers = []
                for b in (0, 1):
                    gs = [0, lens[0]][b]
                    ge = gs + lens[b]
                    a = max(w0, gs)
                    e_ = min(w0 + wl, ge)
                    if e_ > a:
                        gathers.append((round_base + a, e_ - a, b))
                calls[jj].append((round_base + w0, wl, gathers))
                w0 += wl
    tot = pos

    # fill per-core streams
    # position of each edge: seg_base[seg] + occurrence rank within (core, seg)
    comb2 = core * n_seg + seg_of_edge
    order2 = np.argsort(comb2, kind="stable")
    within = np.empty(E, np.int64)
    within[order2] = _occurrence_rank(comb2[order2])
    epos = seg_base[seg_of_edge] + within

    gstream = np.zeros((C, tot), np.int16)                  # pad: bank row 0
    sstream = np.full((C, tot), cfg.CH_ROWS, np.int16)      # pad: dummy row
    gstream[core, epos] = row_in_bank.astype(np.int16)
    sstream[core, epos] = scat_local.astype(np.int16)
    return calls, tot, gstream, sstream


def _idx_layout(stream):
    """[tot] int16 -> [128, tot//16] wrapped+replicated layout."""
    tot = stream.shape[0]
    assert tot % 16 == 0
    base = stream.reshape(tot // 16, 16).T  # [16, tot/16]
    return np.ascontiguousarray(np.tile(base, (8, 1)))


def _build_module(cfg, tot_slots, calls):
    import concourse.bacc as bacc
    import concourse.mybir as mybir
    from concourse import tile

    f32 = mybir.dt.float32
    i16 = mybir.dt.int16
    AF = mybir.ActivationFunctionType
    ALU = mybir.AluOpType

    nc = bacc.Bacc(
        "TRN2", target_bir_lowering=False, debug=False, num_devices=cfg.cores
    )

    # ---- I/O ----
    rcT_d = nc.dram_tensor("rcT", [H, cfg.NCp], f32, kind="ExternalInput")
    xv_d = nc.dram_tensor("xv", [cfg.NVp, H], f32, kind="ExternalInput")
    gidx_d = nc.dram_tensor("gidx", [128, tot_slots // 16], i16, kind="ExternalInput")
    sidx_d = nc.dram_tensor("sidx", [128, tot_slots // 16], i16, kind="ExternalInput")
    W1_d = nc.dram_tensor("W1", [H, H], f32, kind="ExternalInput")
    b1_d = nc.dram_tensor("b1", [H, 1], f32, kind="ExternalInput")
    W2_d = nc.dram_tensor("W2", [H, 4 * H], f32, kind="ExternalInput")
    g1_d = nc.dram_tensor("g1b", [128, 4 * H], f32, kind="ExternalInput")
    bt1_d = nc.dram_tensor("bt1b", [128, 4 * H], f32, kind="ExternalInput")
    W3_d = nc.dram_tensor("W3", [H, H], f32, kind="ExternalInput")
    b3_d = nc.dram_tensor("b3", [H, 1], f32, kind="ExternalInput")
    W4_d = nc.dram_tensor("W4", [H, H], f32, kind="ExternalInput")
    g2_d = nc.dram_tensor("g2b", [128, H], f32, kind="ExternalInput")
    bt2_d = nc.dram_tensor("bt2b", [128, H], f32, kind="ExternalInput")
    id_d = nc.dram_tensor("ident", [128, 128], f32, kind="ExternalInput")
    out_d = nc.dram_tensor("out", [cfg.NVp, H], f32, kind="ExternalOutput")

    # ---- internal DRAM ----
    m_dram = nc.dram_tensor("m_scratch", [2, cfg.BANK_ALLOC, H], f32)
    partial = [
        nc.dram_tensor(f"partial{j}", [cfg.CH_ALLOC, H], f32) for j in range(cfg.J)
    ]
    rs_all = nc.dram_tensor("rs_all", [cfg.NVp, H], f32)

    rg = [list(range(cfg.cores))]

    with tile.TileContext(nc) as tc:
        with tc.tile_pool(name="consts", bufs=1) as cp:
            W1_s = cp.tile([H, H], f32)
            nc.sync.dma_start(W1_s[:], W1_d[:])
            b1_s = cp.tile([H, 1], f32)
            nc.sync.dma_start(b1_s[:], b1_d[:])
            W2_s = cp.tile([H, 4 * H], f32)
            nc.sync.dma_start(W2_s[:], W2_d[:])
            g1_s = cp.tile([128, 4 * H], f32)
            nc.sync.dma_start(g1_s[:], g1_d[:])
            bt1_s = cp.tile([128, 4 * H], f32)
            nc.sync.dma_start(bt1_s[:], bt1_d[:])
            W3_s = cp.tile([H, H], f32)
            nc.sync.dma_start(W3_s[:], W3_d[:])
            b3_s = cp.tile([H, 1], f32)
            nc.sync.dma_start(b3_s[:], b3_d[:])
            W4_s = cp.tile([H, H], f32)
            nc.sync.dma_start(W4_s[:], W4_d[:])
            g2_s = cp.tile([128, H], f32)
            nc.sync.dma_start(g2_s[:], g2_d[:])
            bt2_s = cp.tile([128, H], f32)
            nc.sync.dma_start(bt2_s[:], bt2_d[:])
            id_s = cp.tile([128, 128], f32)
            nc.sync.dma_start(id_s[:], id_d[:])

            zero_s = cp.tile([128, 2048], f32)
            nc.vector.memset(zero_s[:], 0.0)
            eps_s = cp.tile([128, 1], f32)
            nc.vector.memset(eps_s[:], EPS)

            # ---- zero the partial accumulators, m pad rows, rs pad rows ----
            for j in range(cfg.J):
                base = 0
                while base < cfg.CH_ALLOC:
                    rows = min(2048, cfg.CH_ALLOC - base)
                    dstv = partial[j][base : base + rows, :].rearrange(
                        "(p a) h -> p (a h)", p=128
                    )
                    nc.sync.dma_start(dstv, zero_s[:, : rows * H // 128])
                    base += rows
            for s in range(2):
                nc.sync.dma_start(
                    m_dram[s, cfg.BANK_ROWS : cfg.BANK_ROWS + 128, :],
                    zero_s[:, :H],
                )
            if cfg.NVp > cfg.NVs:
                nc.sync.dma_start(
                    rs_all[cfg.NVs : cfg.NVp, :], zero_s[: cfg.NVp - cfg.NVs, :H]
                )

            # ================= Phase 1: cst_send MLP =================
            with (
                tc.tile_pool(name="m1_io", bufs=3) as iop,
                tc.tile_pool(name="m1_mid", bufs=3) as midp,
                tc.tile_pool(name="m1_stat", bufs=3) as stp,
                tc.tile_pool(name="ps1", bufs=2, space="PSUM") as psA,
                tc.tile_pool(name="ps2", bufs=2, space="PSUM") as psB,
            ):
                for t in range(cfg.T1):
                    xt = iop.tile([128, 128], f32, tag="xt")
                    nc.sync.dma_start(xt[:], rcT_d[:, t * 128 : (t + 1) * 128])
                    ps_h1 = psA.tile([128, 128], f32, tag="ps_h1")
                    nc.tensor.matmul(ps_h1[:], W1_s[:], xt[:], start=True, stop=True)
                    h1 = midp.tile([128, 128], f32, tag="h1")
                    nc.scalar.activation(h1[:], ps_h1[:], AF.Relu, bias=b1_s[:])
                    ps_m = psB.tile([128, 512], f32, tag="ps_m")
                    nc.tensor.matmul(ps_m[:], h1[:], W2_s[:], start=True, stop=True)

                    # LayerNorm over 512 (free dim)
                    sq = midp.tile([128, 512], f32, tag="sq")
                    s1 = stp.tile([128, 1], f32, tag="s1")
                    s2 = stp.tile([128, 1], f32, tag="s2")
                    nc.scalar.activation(sq[:], ps_m[:], AF.Identity, accum_out=s1[:])
                    nc.scalar.activation(sq[:], ps_m[:], AF.Square, accum_out=s2[:])
                    mu = stp.tile([128, 1], f32, tag="mu")
                    nc.vector.tensor_scalar(mu[:], s1[:], 1.0 / 512, None, ALU.mult)
                    var = stp.tile([128, 1], f32, tag="var")
                    nc.vector.tensor_scalar(var[:], s2[:], 1.0 / 512, None, ALU.mult)
                    mu2 = stp.tile([128, 1], f32, tag="mu2")
                    nc.vector.tensor_tensor(mu2[:], mu[:], mu[:], ALU.mult)
                    nc.vector.tensor_tensor(var[:], var[:], mu2[:], ALU.subtract)
                    sd = stp.tile([128, 1], f32, tag="sd")
                    nc.scalar.activation(sd[:], var[:], AF.Sqrt, bias=eps_s[:])
                    rsig = stp.tile([128, 1], f32, tag="rsig")
                    nc.vector.reciprocal(rsig[:], sd[:])
                    nmr = stp.tile([128, 1], f32, tag="nmr")
                    nc.vector.tensor_scalar(
                        nmr[:], mu[:], rsig[:], -1.0, ALU.mult, ALU.mult
                    )
                    mn = midp.tile([128, 512], f32, tag="mn")
                    nc.scalar.activation(
                        mn[:], ps_m[:], AF.Identity, bias=nmr[:], scale=rsig[:]
                    )
                    nc.vector.tensor_tensor(mn[:], mn[:], g1_s[:], ALU.mult)
                    nc.vector.tensor_tensor(mn[:], mn[:], bt1_s[:], ALU.add)

                    bank = 0 if t < cfg.BANK_TILES else 1
                    row = (t - bank * cfg.BANK_TILES) * 512
                    dstv = m_dram[bank, row : row + 512, :].rearrange(
                        "(c s) h -> c (s h)", c=128
                    )
                    nc.sync.dma_start(dstv, mn[:])

            # ================= Phase 2: gather / scatter-add =================
            with (
                tc.tile_pool(name="slots", bufs=3) as sp,
                tc.tile_pool(name="idxp", bufs=6) as ip,
            ):
                for j in range(cfg.J):
                    for slot0, wl, gathers in calls[j]:
                        st = sp.tile([128, cfg.sub // 128, 128], f32, tag="slots")
                        for gs0, glen, b in gathers:
                            it = ip.tile([128, cfg.sub // 16], i16, tag="gi")
                            nc.sync.dma_start(
                                it[:, : glen // 16],
                                gidx_d[:, gs0 // 16 : (gs0 + glen) // 16],
                            )
                            c0 = (gs0 - slot0) // 128
                            nc.gpsimd.dma_gather(
                                st[:, c0 : c0 + glen // 128, :],
                                m_dram[b],
                                it[:, : glen // 16],
                                glen,
                                glen,
                                H,
                            )
                        si = ip.tile([128, cfg.sub // 16], i16, tag="si")
                        nc.sync.dma_start(
                            si[:, : wl // 16],
                            sidx_d[:, slot0 // 16 : (slot0 + wl) // 16],
                        )
                        nc.gpsimd.dma_scatter_add(
                            partial[j][:, :],
                            st[:, : wl // 128, :],
                            si[:, : wl // 16],
                            wl,
                            wl,
                            H,
                        )
                    nc.gpsimd.collective_compute(
                        "ReduceScatter",
                        mybir.AluOpType.add,
                        replica_groups=rg,
                        ins=[partial[j][0 : cfg.CH_ROWS, :]],
                        outs=[rs_all[j * cfg.S : (j + 1) * cfg.S, :]],
                    )

            # ================= Phase 3: val_rec MLP =================
            with (
                tc.tile_pool(name="m2_io", bufs=3) as iop2,
                tc.tile_pool(name="m2_mid", bufs=3) as midp2,
                tc.tile_pool(name="m2_stat", bufs=3) as stp2,
                tc.tile_pool(name="ps1b", bufs=2, space="PSUM") as psA,
                tc.tile_pool(name="ps2b", bufs=2, space="PSUM") as psB,
            ):
                for t in range(cfg.T2):
                    xt2 = iop2.tile([128, 128], f32, tag="xt2")
                    nc.sync.dma_start(xt2[:], xv_d[t * 128 : (t + 1) * 128, :])
                    rv = iop2.tile([128, 128], f32, tag="rv")
                    nc.sync.dma_start(rv[:], rs_all[t * 128 : (t + 1) * 128, :])
                    hh = midp2.tile([128, 128], f32, tag="hh")
                    nc.vector.tensor_tensor(hh[:], xt2[:], rv[:], ALU.add)
                    ps_hT = psA.tile([128, 128], f32, tag="ps_hT")
                    nc.tensor.transpose(ps_hT[:], hh[:], id_s[:])
                    hT = midp2.tile([128, 128], f32, tag="hT")
                    nc.scalar.copy(hT[:], ps_hT[:])
                    ps_h2 = psA.tile([128, 128], f32, tag="ps_h2")
                    nc.tensor.matmul(ps_h2[:], W3_s[:], hT[:], start=True, stop=True)
                    h2 = midp2.tile([128, 128], f32, tag="h2")
                    nc.scalar.activation(h2[:], ps_h2[:], AF.Relu, bias=b3_s[:])
                    ps_o = psB.tile([128, 128], f32, tag="ps_o")
                    nc.tensor.matmul(ps_o[:], h2[:], W4_s[:], start=True, stop=True)

                    sq2 = midp2.tile([128, 128], f32, tag="sq2")
                    s1b = stp2.tile([128, 1], f32, tag="s1b")
                    s2b = stp2.tile([128, 1], f32, tag="s2b")
                    nc.scalar.activation(sq2[:], ps_o[:], AF.Identity, accum_out=s1b[:])
                    nc.scalar.activation(sq2[:], ps_o[:], AF.Square, accum_out=s2b[:])
                    mu_b = stp2.tile([128, 1], f32, tag="mu_b")
                    nc.vector.tensor_scalar(mu_b[:], s1b[:], 1.0 / H, None, ALU.mult)
                    var_b = stp2.tile([128, 1], f32, tag="var_b")
                    nc.vector.tensor_scalar(var_b[:], s2b[:], 1.0 / H, None, ALU.mult)
                    mu2b = stp2.tile([128, 1], f32, tag="mu2b")
                    nc.vector.tensor_tensor(mu2b[:], mu_b[:], mu_b[:], ALU.mult)
                    nc.vector.tensor_tensor(var_b[:], var_b[:], mu2b[:], ALU.subtract)
                    sdb = stp2.tile([128, 1], f32, tag="sdb")
                    nc.scalar.activation(sdb[:], var_b[:], AF.Sqrt, bias=eps_s[:])
                    rsb = stp2.tile([128, 1], f32, tag="rsb")
                    nc.vector.reciprocal(rsb[:], sdb[:])
                    nmrb = stp2.tile([128, 1], f32, tag="nmrb")
                    nc.vector.tensor_scalar(
                        nmrb[:], mu_b[:], rsb[:], -1.0, ALU.mult, ALU.mult
                    )
                    on = midp2.tile([128, 128], f32, tag="on")
                    nc.scalar.activation(
                        on[:], ps_o[:], AF.Identity, bias=nmrb[:], scale=rsb[:]
                    )
                    nc.vector.tensor_tensor(on[:], on[:], g2_s[:], ALU.mult)
                    nc.vector.tensor_tensor(on[:], on[:], bt2_s[:], ALU.add)
                    nc.vector.tensor_tensor(on[:], on[:], xt2[:], ALU.add)
                    nc.sync.dma_start(out_d[t * 128 : (t + 1) * 128, :], on[:])

    nc.compile()
    return nc


def _prep_inputs(cfg, inputs):
    """Host-side sharding; returns (in_maps, tot_slots, calls)."""
    x_val = np.ascontiguousarray(np.asarray(inputs["x_val"], np.float32))
    r_cst = np.ascontiguousarray(np.asarray(inputs["r_cst"], np.float32))
    edges = np.asarray(inputs["cst_edges"]).astype(np.int64)
    le = np.asarray(inputs["LE"]).astype(np.int64)
    pe = np.asarray(inputs["PE"]).astype(np.int64)
    slot = 2 * le + pe

    calls, tot, gstream, sstream = _build_plan(cfg, edges[0], edges[1], slot)
    tot = max(tot, 128)

    W1 = np.asarray(inputs["W1"], np.float32)
    b1 = np.asarray(inputs["b1"], np.float32).reshape(H, 1)
    W2 = np.asarray(inputs["W2"], np.float32)
    g1b = np.ascontiguousarray(
        np.broadcast_to(np.asarray(inputs["g1"], np.float32), (128, 4 * H))
    )
    bt1b = np.ascontiguousarray(
        np.broadcast_to(np.asarray(inputs["bt1"], np.float32), (128, 4 * H))
    )
    W3 = np.asarray(inputs["W3"], np.float32)
    b3 = np.asarray(inputs["b3"], np.float32).reshape(H, 1)
    W4 = np.asarray(inputs["W4"], np.float32)
    g2b = np.ascontiguousarray(
        np.broadcast_to(np.asarray(inputs["g2"], np.float32), (128, H))
    )
    bt2b = np.ascontiguousarray(
        np.broadcast_to(np.asarray(inputs["bt2"], np.float32), (128, H))
    )
    ident = np.eye(128, dtype=np.float32)

    in_maps = []
    for c in range(cfg.cores):
        rc = r_cst[c * cfg.NCs : (c + 1) * cfg.NCs]
        rcT = np.zeros((H, cfg.NCp), np.float32)
        rcT[:, : cfg.NCs] = rc.T
        xv = np.zeros((cfg.NVp, H), np.float32)
        xv[: cfg.NVs] = x_val[c * cfg.NVs : (c + 1) * cfg.NVs]
        gs = np.zeros(tot, np.int16)
        ss = np.full(tot, cfg.CH_ROWS, np.int16)
        gs[: gstream.shape[1]] = gstream[c]
        ss[: sstream.shape[1]] = sstream[c]
        in_maps.append(
            {
                "rcT": np.ascontiguousarray(rcT),
                "xv": xv,
                "gidx": _idx_layout(gs),
                "sidx": _idx_layout(ss),
                "W1": W1,
                "b1": b1,
                "W2": W2,
                "g1b": g1b,
                "bt1b": bt1b,
                "W3": W3,
                "b3": b3,
                "W4": W4,
                "g2b": g2b,
                "bt2b": bt2b,
                "ident": ident,
            }
        )
    return in_maps, tot, calls


def run(inputs, cfg=None, trace=False):
    """Build, run on hardware, return (output, BassKernelResults)."""
    from concourse.bass_utils import run_bass_kernel_spmd

    cfg = cfg or Cfg()
    in_maps, tot, calls = _prep_inputs(cfg, inputs)
    nc = _build_module(cfg, tot, calls)
    res = run_bass_kernel_spmd(
        nc, in_maps, core_ids=list(range(cfg.cores)), trace=trace
    )
    out = np.concatenate(
        [res.results[c]["out"][: cfg.NVs] for c in range(cfg.cores)], axis=0
    )
    return out, res


def kernel(**inputs) -> np.ndarray:
    out, _ = run(inputs)
    return out

